# revision 13
# baseline (speedup 1.0000x reference)
"""MoE (DeepSeek-style) routed+shared expert forward on 8 TRN2 NeuronCores.

Strategy (expert-parallel, host-side dispatch):
  - Host computes the gate (softmax + top-2) in float64 and gathers each
    expert's routed tokens (padded to a uniform capacity C2).
  - Core e runs expert e's routed tokens through the SwiGLU FFN in
    fp8(e4m3) with DoubleRow matmuls (2 k-tiles per instruction), plus a
    1/8 slice of all tokens through the replicated shared-expert MLP in
    fp16.
  - Weights are scaled by 128 into e4m3 on the host; the activation
    instructions de-scale via their `scale` operand. h is stored fp8
    scaled by 8 so the w2 matmul also runs in DoubleRow mode.
  - Column blocks are streamed per weight load (all NB blocks reuse one
    LDWEIGHTS) to amortize the DoubleRow weight-load penalty.
  - Host scatters expert outputs back by routing index, scales by the
    gate weights, and adds the shared-expert output.
"""

import sys

if "/opt/trn_rl_repo" not in sys.path:
    sys.path.insert(0, "/opt/trn_rl_repo")

import ml_dtypes
import numpy as np

import concourse.bass as bass
import concourse.tile as tile
from concourse import bacc, mybir
from concourse import bass_utils

B, S, DIM = 4, 2048, 1024
T = B * S
INTER = 1024
E = 8
TOPK = 2
ROUTE_SCALE = 1.0
SHARED_INTER = 2048
N_CORES = 8
TS = T // N_CORES   # shared-expert tokens per core
SW = 128.0          # weight scale into e4m3
SHS = 8.0           # h scale into e4m3

F32 = mybir.dt.float32
F16 = mybir.dt.float16
F8 = mybir.dt.float8e4
SILU = mybir.ActivationFunctionType.Silu
IDENT = mybir.ActivationFunctionType.Identity
DR = mybir.MatmulPerfMode.DoubleRow

_program_cache = {}


def build_program(C2, BS, NB):
    """Per-core SPMD Bass program. C2 = NB*BS routed capacity."""
    assert 2 <= NB <= 8, f"dr_chain streams NB blocks over an 8-bank PSUM ring, {NB=}"
    nc = bacc.Bacc("TRN2", target_bir_lowering=False, debug=False,
                   num_devices=N_CORES)

    def din(name, shape, dt):
        return nc.dram_tensor(name, shape, dt, kind="ExternalInput").ap()

    def dout(name, shape, dt):
        return nc.dram_tensor(name, shape, dt, kind="ExternalOutput").ap()

    ND = DIM // 128           # 8 k-tiles over DIM
    NI = INTER // 128
    NS = SHARED_INTER // 128  # 16
    NP = ND // 2              # k-tile pairs for DoubleRow

    # All inputs are host-packed in SBUF layout (partition dim first,
    # per-partition data contiguous) so each DMA is 128 large contiguous
    # descriptors instead of thousands of sub-KB ones.
    xe_r = din("xe8", (128, NB, ND, BS), F8)   # routed tokens
    w1_r = din("w1t8", (128, NP, 2, INTER), F8)
    w3_r = din("w3t8", (128, NP, 2, INTER), F8)
    w2_r = din("w2t8", (128, NP, 2, DIM), F8)
    xs_r = din("xs", (128, ND, TS), F16)       # shared-token slice
    ws1_r = din("ws1t", (128, ND, SHARED_INTER), F16)
    ws3_r = din("ws3t", (128, ND, SHARED_INTER), F16)
    ws2_r = din("ws2t", (128, NS, DIM), F16)
    biases = din("biases", (128, 64), F32)     # host-packed per-partition
    ye = dout("ye", (DIM, C2), F16)
    ys = dout("ys", (DIM, TS), F16)

    ye_r = ye.rearrange("(md p) c -> p md c", p=128)
    ys_r = ys.rearrange("(md p) c -> p md c", p=128)

    with tile.TileContext(nc) as tc:
        from contextlib import ExitStack
        es1 = ExitStack()
        with tc.tile_pool(name="bias", bufs=1) as bpool, \
             tc.tile_pool(name="wsh", bufs=1, side="right") as wspool, \
             tc.tile_pool(name="tmp", bufs=NB + 2) as tpool, \
             tc.tile_pool(name="yout", bufs=NB + 2) as ypool, \
             tc.tile_pool(name="ps", bufs=8, space="PSUM") as pspool:

            wpool = es1.enter_context(tc.tile_pool(name="wexp", bufs=1))
            xpool = es1.enter_context(tc.tile_pool(name="xep", bufs=1))
            hpool = es1.enter_context(tc.tile_pool(name="h8p", bufs=1))

            # ---- PE pre-warm: the HAM clock gate holds the PE at 1.2 GHz
            # until ~3.4us of sustained activity.  Run dummy matmuls on a
            # memset tile while the first real inputs stream in, so real
            # matmuls start at 2.4 GHz. ----
            warm = bpool.tile([128, 640], F8, tag="warm")
            nc.vector.memset(warm[:], 0)
            for i in range(52):
                pw = pspool.tile([128, 512], F32, tag="ps", name="ps",
                                 padded_shape=[128, 512])
                n = 512 if i < 12 else 128
                nc.tensor.matmul(pw[:, 0:n], warm[:, 0:128],
                                 warm[:, 128:128 + n],
                                 start=True, stop=True)

            # ---- phase-1 input DMAs (needed first; issue in PE order) ----
            ball = bpool.tile([128, 64], F32, tag="biases")
            b1_sb = ball[:, 0:NI]
            b3_sb = ball[:, NI:2 * NI]            # pre-scaled by SHS on host
            b2_sb = ball[:, 2 * NI:2 * NI + ND]
            bs1_sb = ball[:, 24:24 + NS]
            bs3_sb = ball[:, 24 + NS:24 + 2 * NS]
            bs2_sb = ball[:, 24 + 2 * NS:24 + 2 * NS + ND]

            w3_sb, w1_sb, w2_sb, xe_sb = [], [], [], []
            t = wpool.tile([128, 2, INTER], F8, tag="w3_0", name="w3_0")
            nc.sync.dma_start(t[:], w3_r[:, 0])
            w3_sb.append(t)
            for b in range(NB):
                t = xpool.tile([128, ND, BS], F8, tag=f"xe{b}", name=f"xe{b}")
                nc.sync.dma_start(t[:], xe_r[:, b])
                xe_sb.append(t)
                if b < NP - 1:
                    t = wpool.tile([128, 2, INTER], F8, tag=f"w3_{b + 1}",
                                   name=f"w3_{b + 1}")
                    nc.sync.dma_start(t[:], w3_r[:, b + 1])
                    w3_sb.append(t)
            for j in range(len(w3_sb), NP):
                t = wpool.tile([128, 2, INTER], F8, tag=f"w3_{j}",
                               name=f"w3_{j}")
                nc.sync.dma_start(t[:], w3_r[:, j])
                w3_sb.append(t)
            for j in range(NP):
                t = wpool.tile([128, 2, INTER], F8, tag=f"w1_{j}")
                nc.sync.dma_start(t[:], w1_r[:, j])
                w1_sb.append(t)
            for j in range(NP):
                t = wpool.tile([128, 2, DIM], F8, tag=f"w2_{j}")
                nc.sync.dma_start(t[:], w2_r[:, j])
                w2_sb.append(t)
            nc.sync.dma_start(ball[:], biases[:])

            # ---- phase-2 weights: DMA streams during phase-1 compute ----
            xs_sb = wspool.tile([128, ND, TS], F16, tag="xs")
            nc.sync.dma_start(xs_sb[:], xs_r[:])
            ws3_sb = wspool.tile([128, ND, SHARED_INTER], F16, tag="ws3")
            nc.sync.dma_start(ws3_sb[:], ws3_r[:])
            ws1_sb = wspool.tile([128, ND, SHARED_INTER], F16, tag="ws1")
            nc.sync.dma_start(ws1_sb[:], ws1_r[:])
            ws2_sb = wspool.tile([128, NS, DIM], F16, tag="ws2")
            nc.sync.dma_start(ws2_sb[:], ws2_r[:])

            h_sb = [hpool.tile([128, NI, BS], F8, tag=f"h{b}", name=f"h{b}")
                    for b in range(NB)]

            # ================= Phase 1: routed expert (fp8 DoubleRow) ====
            def dr_chain(w_tiles, msel, rhs_tiles, rhs_of, n):
                """Accumulate NP DoubleRow matmuls into one PSUM tile per
                rhs block, streaming all blocks per weight load."""
                pss = [pspool.tile([128, n], F32, tag="ps", name="ps",
                                   padded_shape=[128, 512])
                       for _ in rhs_tiles]
                for j in range(NP):
                    for bi, rt in enumerate(rhs_tiles):
                        nc.tensor.matmul(
                            pss[bi][:],
                            w_tiles[j][:, :, msel],
                            rhs_of(rt, j),
                            start=(j == 0), stop=(j == NP - 1),
                            perf_mode=DR)
                return pss

            xe_of = lambda rt, j: rt[:, 2 * j:2 * j + 2, :]

            for mi in range(NI):
                msel = slice(mi * 128, (mi + 1) * 128)
                ps3 = dr_chain(w3_sb, msel, xe_sb, xe_of, BS)
                t3s = []
                for b in range(NB):
                    t3 = tpool.tile([128, BS], F16, tag="t3",
                                    padded_shape=[128, 512])
                    nc.scalar.activation(t3[:], ps3[b][:], IDENT,
                                         bias=b3_sb[:, mi:mi + 1],
                                         scale=SHS / SW)
                    t3s.append(t3)
                ps1 = dr_chain(w1_sb, msel, xe_sb, xe_of, BS)
                for b in range(NB):
                    t1 = tpool.tile([128, BS], F16, tag="t1",
                                    padded_shape=[128, 512])
                    nc.scalar.activation(t1[:], ps1[b][:], SILU,
                                         bias=b1_sb[:, mi:mi + 1],
                                         scale=1.0 / SW)
                    nc.vector.tensor_mul(h_sb[b][:, mi, :], t1[:], t3s[b][:])

            h_of = lambda rt, j: rt[:, 2 * j:2 * j + 2, :]
            for md in range(ND):
                msel = slice(md * 128, (md + 1) * 128)
                psy = dr_chain(w2_sb, msel, h_sb, h_of, BS)
                for b in range(NB):
                    yt = ypool.tile([128, BS], F16, tag="yt",
                                    padded_shape=[128, 512])
                    nc.scalar.activation(yt[:], psy[b][:], IDENT,
                                         bias=b2_sb[:, md:md + 1],
                                         scale=1.0 / (SW * SHS))
                    nc.sync.dma_start(ye_r[:, md, b * BS:(b + 1) * BS], yt[:])

            es1.close()  # free phase-1 pools; hsp reuses their space

            # ================= Phase 2: shared expert (fp16) =============
            NBS = 2
            BSS = TS // NBS  # 512
            hspool = ExitStack()
            hsp = hspool.enter_context(tc.tile_pool(name="hsp", bufs=1))
            hs_sb = [hsp.tile([128, NS, BSS], F16, tag=f"hs{b}", name=f"hs{b}")
                     for b in range(NBS)]

            def f16_chain(w_sb, msel, x_sb, nk, n):
                pss = [pspool.tile([128, n], F32, tag="ps", name="ps",
                                   padded_shape=[128, 512])
                       for _ in range(NBS)]
                for k in range(nk):
                    for b in range(NBS):
                        nc.tensor.matmul(
                            pss[b][:],
                            w_sb[:, k, msel],
                            x_sb[:, k, b * n:(b + 1) * n],
                            start=(k == 0), stop=(k == nk - 1))
                return pss

            for mi in range(NS):
                msel = slice(mi * 128, (mi + 1) * 128)
                ps3 = f16_chain(ws3_sb, msel, xs_sb, ND, BSS)
                t3s = []
                for b in range(NBS):
                    t3 = tpool.tile([128, BSS], F16, tag="t3",
                                    padded_shape=[128, 512])
                    nc.scalar.activation(t3[:], ps3[b][:], IDENT,
                                         bias=bs3_sb[:, mi:mi + 1])
                    t3s.append(t3)
                ps1 = f16_chain(ws1_sb, msel, xs_sb, ND, BSS)
                for b in range(NBS):
                    t1 = tpool.tile([128, BSS], F16, tag="t1",
                                    padded_shape=[128, 512])
                    nc.scalar.activation(t1[:], ps1[b][:], SILU,
                                         bias=bs1_sb[:, mi:mi + 1])
                    nc.vector.tensor_mul(hs_sb[b][:, mi, :], t1[:], t3s[b][:])

            for md in range(ND):
                msel = slice(md * 128, (md + 1) * 128)
                pss = [pspool.tile([128, BSS], F32, tag="ps", name="ps",
                                   padded_shape=[128, 512])
                       for _ in range(NBS)]
                for ji in range(NS):
                    for b in range(NBS):
                        nc.tensor.matmul(
                            pss[b][:],
                            ws2_sb[:, ji, msel],
                            hs_sb[b][:, ji, :],
                            start=(ji == 0), stop=(ji == NS - 1))
                for b in range(NBS):
                    yt = ypool.tile([128, BSS], F16, tag="yt",
                                    padded_shape=[128, 512])
                    nc.scalar.activation(yt[:], pss[b][:], IDENT,
                                         bias=bs2_sb[:, md:md + 1])
                    nc.sync.dma_start(ys_r[:, md, b * BSS:(b + 1) * BSS],
                                      yt[:])
            hspool.close()

    nc.compile()
    return nc


def _pack_biases(b1, b3, b2, bs1, bs3, bs2):
    """Pack all bias vectors into one [128, 64] per-partition table.
    b3 is pre-scaled by SHS (its activation writes SHS*z3)."""
    out = np.zeros((128, 64), np.float32)
    cols = [(b1, 0), (b3 * SHS, 8), (b2, 16), (bs1, 24), (bs3, 40),
            (bs2, 56)]
    for vec, c0 in cols:
        k = len(vec) // 128
        out[:, c0:c0 + k] = vec.reshape(k, 128).T
    return out


def _gate_host(xt, gate_w, gate_b):
    """Softmax gate + top-2 routing, computed in float64 on the host."""
    logits = xt.astype(np.float64) @ gate_w.astype(np.float64).T \
        + gate_b.astype(np.float64)
    m = logits.max(axis=-1, keepdims=True)
    p = np.exp(logits - m)
    scores = p / p.sum(axis=-1, keepdims=True)
    order = np.argsort(-scores, axis=1, kind="stable")
    top_i = order[:, :TOPK]
    top_w = (np.take_along_axis(scores, top_i, axis=1)
             * ROUTE_SCALE).astype(np.float32)
    return top_i, top_w


def run(inputs, trace=False):
    f8 = ml_dtypes.float8_e4m3   # TRN-style e4m3 (max 240)
    f16 = np.float16

    x = np.ascontiguousarray(np.asarray(inputs["x"], dtype=np.float32))
    gate_w = np.asarray(inputs["gate_w"], dtype=np.float32)
    gate_b = np.asarray(inputs["gate_b"], dtype=np.float32)
    w1 = np.asarray(inputs["w1"], dtype=np.float32)
    b1 = np.asarray(inputs["b1"], dtype=np.float32)
    w3 = np.asarray(inputs["w3"], dtype=np.float32)
    b3 = np.asarray(inputs["b3"], dtype=np.float32)
    w2 = np.asarray(inputs["w2"], dtype=np.float32)
    b2 = np.asarray(inputs["b2"], dtype=np.float32)
    ws1 = np.asarray(inputs["ws1"], dtype=np.float32)
    bs1 = np.asarray(inputs["bs1"], dtype=np.float32)
    ws3 = np.asarray(inputs["ws3"], dtype=np.float32)
    bs3 = np.asarray(inputs["bs3"], dtype=np.float32)
    ws2 = np.asarray(inputs["ws2"], dtype=np.float32)
    bs2 = np.asarray(inputs["bs2"], dtype=np.float32)

    xt = x.reshape(T, DIM)
    top_i, top_w = _gate_host(xt, gate_w, gate_b)

    # Dispatch: token lists + gate weights per expert.
    idx, wgt = [], []
    for e in range(E):
        toks = np.nonzero((top_i == e).any(axis=1))[0]
        idx.append(toks)
        slot = (top_i[toks] == e)            # [n_e, TOPK], one True per row
        wgt.append(top_w[toks][slot])

    cmax = max(max(len(i) for i in idx), 256)
    NB = max(2, -(-cmax // 512))             # blocks per weight load
    BS = -(-cmax // (NB * 16)) * 16          # block size, multiple of 16
    C2 = NB * BS

    ND, NP, NS = DIM // 128, DIM // 256, SHARED_INTER // 128

    def pack_w(wt, rows):
        # [rows, cols] -> [128, NP, 2, cols] with partition-contiguous rows
        return np.ascontiguousarray(
            wt.reshape(rows // 256, 2, 128, wt.shape[1]).transpose(2, 0, 1, 3))

    def pack_x(xt_, k):
        # [rows, cols] -> [128, k, cols]
        return np.ascontiguousarray(
            xt_.reshape(k, 128, xt_.shape[1]).transpose(1, 0, 2))

    ws1t = pack_x(ws1.T.astype(f16), ND)
    ws3t = pack_x(ws3.T.astype(f16), ND)
    ws2t = pack_x(ws2.T.astype(f16), NS)

    in_maps = []
    for e in range(E):
        xe = np.zeros((C2, DIM), np.float32)
        xe[:len(idx[e])] = xt[idx[e]]
        # [C2, DIM] -> [128, NB, ND, BS]
        xe8 = np.ascontiguousarray(
            xe.reshape(NB, BS, ND, 128).transpose(3, 0, 2, 1)).astype(f8)
        sl = slice(TS * e, TS * (e + 1))
        in_maps.append({
            "xe8": xe8,
            "xs": pack_x(np.ascontiguousarray(xt[sl].T).astype(f16), ND),
            "w1t8": pack_w((w1[e] * SW).T.astype(f8), DIM),
            "w3t8": pack_w((w3[e] * SW).T.astype(f8), DIM),
            "w2t8": pack_w((w2[e] * SW).T.astype(f8), INTER),
            "ws1t": ws1t, "ws3t": ws3t, "ws2t": ws2t,
            "biases": _pack_biases(b1[e], b3[e], b2[e], bs1, bs3, bs2),
        })

    key = (C2, BS, NB)
    if key not in _program_cache:
        _program_cache[key] = build_program(C2, BS, NB)
    nc = _program_cache[key]

    res = bass_utils.run_bass_kernel_spmd(
        nc, in_maps, core_ids=list(range(N_CORES)), trace=trace)

    y = np.empty((T, DIM), np.float32)
    for e in range(E):
        sl = slice(TS * e, TS * (e + 1))
        y[sl] = res.results[e]["ys"].T.astype(np.float32)
    for e in range(E):
        yef = res.results[e]["ye"].astype(np.float32)
        y[idx[e]] += yef[:, :len(idx[e])].T * wgt[e][:, None]
    return y.reshape(B, S, DIM), res


def kernel(**inputs) -> np.ndarray:
    out, _ = run(inputs, trace=False)
    return out
